# revision 31
# baseline (speedup 1.0000x reference)
"""Causal self-attention (GQA + RMS-norm + partial RoPE) Trainium2 kernel.

Full inputs in, full output out. Sharding: 8 cores = batch(4) x head-half(2).
Each core handles one batch and 8 q-heads / 2 kv-heads in transposed layouts
(head_dim on partitions). v3 design:

- All matmul operands bf16 (fp32 PSUM accumulate): fast weight load, DVE 2x
  modes, halved SBUF/DMA. Weights DMAed once and kept resident.
- Single activation table set (natural_log_exp_and_others): Exp, Ln, Square,
  Copy only; rsqrt(x) = exp(-0.5*ln(x)). Table chooser pinned to that set.
- R / 1/l broadcasts via gpsimd.partition_broadcast (no tensor-engine
  broadcast matmuls, no PSUM banks for them).
- Attention: key chunks in PAIRS ([128,2,512] PSUM, one Exp per pair);
  TWO heads interleaved so one head's exp/mask latency hides behind the
  other head's matmuls. Causal mask: one gpsimd affine_select per
  diagonal pair.
- PSUM banks: pA/pB [128,2,512] (2 each) + acc0/acc1 [128,512] +
  vec1a/vec1b [1,512] = 8.
"""
import numpy as np
import ml_dtypes

import concourse.bacc as bacc
import concourse.mybir as mybir
import concourse.bass_isa as bass_isa
from concourse.tile import TileContext
from concourse.bass_utils import run_bass_kernel_spmd

# The ACT table-load inserter picks the FIRST act-function set covering each
# activation: Exp/Square/Copy -> exp_and_others(0) but Ln -> natural_log(5),
# so interleaved norm+softmax work thrashes table loads (~1.3us each).  All
# four functions we use coexist in natural_log_exp_and_others; steer the
# chooser there by hiding them from the coverage sets of every OTHER table.
# Set ids are unchanged, so the emitted program stays valid.
_AF = mybir.ActivationFunctionType
_PINNED_SET = "natural_log_exp_and_others"
_PINNED_FUNCS = {_AF.Exp, _AF.Ln, _AF.Square, _AF.Copy}
_orig_get_act_tables = bacc.get_activation_tables


def _pinned_get_act_tables(arch):
    tabs = _orig_get_act_tables(arch)
    return {
        name: (funcs if name == _PINNED_SET else funcs - _PINNED_FUNCS)
        for name, funcs in tabs.items()
    }


bacc.get_activation_tables = _pinned_get_act_tables

F32 = mybir.dt.float32
F32R = mybir.dt.float32r
BF16 = mybir.dt.bfloat16
AF = mybir.ActivationFunctionType

B, S, D = 4, 2048, 2048
H, KV, HD = 16, 4, 128
ROPE, HALF_ROPE = 64, 32
EPS = 1.1920929e-07
N_CORES = 8
NDC = D // 128          # 16 contraction chunks
NQC = S // 512          # 4 query chunks of 512
LH = 8                  # local q heads per core
LKV = 2                 # local kv heads per core

_cached_program = None
_last_in_maps = None


def _build_program():
    nc = bacc.Bacc("TRN2")
    t = nc.alloc_sbuf_tensor("const-f32-eps", [128, 1], F32)
    nc.gpsimd.memset(t.ap(), EPS)
    nc.const_aps.aps[(F32, EPS)] = t.ap()
    nc.all_engine_barrier()

    xT = nc.declare_dram_parameter("xT", [D, S], BF16, isOutput=False)
    wqT = nc.declare_dram_parameter("wqT", [D, LH * HD], BF16, isOutput=False)
    wkT = nc.declare_dram_parameter("wkT", [D, LKV * HD], BF16, isOutput=False)
    wvT = nc.declare_dram_parameter("wvT", [D, LKV * HD], BF16, isOutput=False)
    wpT = nc.declare_dram_parameter("wpT", [LH * HD, D], BF16, isOutput=False)
    ccatd = nc.declare_dram_parameter("ccat", [ROPE, S], BF16, isOutput=False)
    scatd = nc.declare_dram_parameter("scat", [ROPE, S], BF16, isOutput=False)
    o128d = nc.declare_dram_parameter("o128", [128, 1], BF16, isOutput=False)
    gaind = nc.declare_dram_parameter("gains", [128, LH], F32, isOutput=False)
    out = nc.declare_dram_parameter("out", [S, D], BF16, isOutput=True)

    with TileContext(nc) as tc:
        with (
            tc.tile_pool(name="cp", bufs=1) as cp,
            tc.tile_pool(name="xap", bufs=2) as xap,
            tc.tile_pool(name="qnp", bufs=2) as qnp,
            tc.tile_pool(name="ytp", bufs=2) as ytp,
            tc.tile_pool(name="scr", bufs=2) as scr,
            tc.tile_pool(name="exp4", bufs=6) as exp4,
            tc.tile_pool(name="scr1", bufs=1) as scr1,
            tc.tile_pool(name="pu", bufs=1, space="PSUM") as pu,
        ):
            # ---- constants / weights, DMA-ordered by first use ----
            o128 = cp.tile([128, 1], BF16, tag="o128")
            nc.sync.dma_start(out=o128[:], in_=o128d[:])
            gains = cp.tile([128, LH], F32, tag="gains")
            nc.sync.dma_start(out=gains[:], in_=gaind[:])
            ccat = cp.tile([ROPE, S], BF16, tag="ccat")
            nc.sync.dma_start(out=ccat[:], in_=ccatd[:])
            scat = cp.tile([ROPE, S], BF16, tag="scat")
            nc.sync.dma_start(out=scat[:], in_=scatd[:])
            wk_t = cp.tile([128, NDC, LKV * HD], BF16, tag="wk")
            xa0 = xap.tile([128, NDC, 512], BF16, tag="xa", name="xa")
            wq_t = cp.tile([128, NDC, LH * HD], BF16, tag="wq")
            wv_t = cp.tile([128, NDC, LKV * HD], BF16, tag="wv")

            def dma_x(xa, pos0):
                for dc in range(NDC):
                    nc.sync.dma_start(
                        out=xa[:, dc],
                        in_=xT[dc * 128:(dc + 1) * 128, pos0:pos0 + 512])

            # startup-only coarse DMAs: nothing competes for SBUF yet,
            # and the ~650ns/DMA issue cost dominates otherwise
            nc.sync.dma_start(out=wk_t[:],
                              in_=wkT.rearrange("(c p) e -> p c e", p=128))
            for q4 in range(4):
                nc.gpsimd.dma_start(
                    out=xa0[:, 4 * q4:4 * (q4 + 1)],
                    in_=xT[512 * q4:512 * (q4 + 1), 0:512]
                    .rearrange("(c p) e -> p c e", p=128))
            for dc in range(NDC):
                nc.sync.dma_start(out=wq_t[:, dc],
                                  in_=wqT[dc * 128:(dc + 1) * 128, :])
            for dc in range(NDC):
                nc.gpsimd.dma_start(out=wv_t[:, dc],
                                    in_=wvT[dc * 128:(dc + 1) * 128, :])
            xa1 = xap.tile([128, NDC, 512], BF16, tag="xa", name="xa")
            dma_x(xa1, 512)
            wp_t = cp.tile([128, LH, D], BF16, tag="wp")
            for hh in range(LH):
                nc.sync.dma_start(out=wp_t[:, hh],
                                  in_=wpT[hh * 128:(hh + 1) * 128, :])
            kn_t = cp.tile([128, LKV, S], BF16, tag="kn")
            v_t = cp.tile([128, S // 128, LKV * HD], BF16, tag="v")
            xa_pre = {0: xa0, 1: xa1}

            # PSUM accumulator rotation: acc0, acc1, plus the (wider) pA/pB
            # slots which the P/O phases may borrow as [128,512] tiles.
            _rot = [0]
            _ROT_TAGS = ["acc0", "acc1", "pA", "pB"]

            def acc_tile(shape=(128, 512), nm="acc"):
                tag = _ROT_TAGS[_rot[0] % 4]
                _rot[0] += 1
                return pu.tile(list(shape), F32, tag=tag, name=nm)

            _v1 = [0]

            def vec1_tile(nm):
                tag = "vec1a" if _v1[0] % 2 == 0 else "vec1b"
                _v1[0] += 1
                return pu.tile([1, 512], F32, tag=tag, name=nm)

            def norm_rope(raw, dst_full, dst_r1, dst_r2, dst_r64, cs):
                """RMS-norm + partial RoPE, raw [128,512] PSUM f32 ->
                bf16 dst (already-allocated APs: full/rows0:32/32:64/0:64).
                cs = column slice into the S-wide rope tables."""
                sq = scr.tile([128, 512], BF16, tag="sq", name="sq")
                nc.scalar.activation(sq[:], raw, AF.Square)
                ssq = vec1_tile("ssq")
                nc.tensor.matmul(ssq[:], o128[:], sq[:], start=True, stop=True)
                lnu = scr1.tile([1, 512], F32, tag="lnu", name="lnu")
                nc.scalar.activation(lnu[:], ssq[:], AF.Ln,
                                     scale=1.0 / HD, bias=EPS)
                rr = scr1.tile([1, 512], F32, tag="rr", name="rr")
                nc.scalar.activation(rr[:], lnu[:], AF.Exp, scale=-0.5)
                Rb = scr.tile([128, 512], F32, tag="Rb", name="Rb")
                nc.gpsimd.partition_broadcast(Rb[:], rr[:])
                nc.vector.tensor_mul(dst_full, raw, Rb[:])
                # scat rows 0:32 = -sin, rows 32:64 = +sin so each TT below
                # has equal SBUF base partitions for its two inputs
                tmp = scr.tile([ROPE, 512], BF16, tag="tmp", name="tmp")
                nc.vector.tensor_mul(tmp[0:HALF_ROPE, :], dst_r2,
                                     scat[HALF_ROPE:ROPE, cs])
                nc.vector.tensor_mul(tmp[HALF_ROPE:ROPE, :], dst_r1,
                                     scat[0:HALF_ROPE, cs])
                nc.vector.tensor_mul(dst_r64, dst_r64, ccat[:, cs])
                nc.vector.tensor_add(dst_r64, dst_r64, tmp[:])

            qn = {}
            _pv = [0]

            def phaseP(sc):
                pos0 = sc * 512
                cs = slice(pos0, pos0 + 512)
                xa = xa_pre.pop(sc)
                def knorm(g, raw_ap):
                    norm_rope(raw_ap, kn_t[:, g, cs],
                              kn_t[0:HALF_ROPE, g, cs],
                              kn_t[HALF_ROPE:ROPE, g, cs],
                              kn_t[0:ROPE, g, cs], cs)

                def qnorm(h, raw_ap):
                    qt = qn[(sc, h)]
                    norm_rope(raw_ap, qt[:], qt[0:HALF_ROPE, :],
                              qt[HALF_ROPE:ROPE, :], qt[0:ROPE, :], cs)

                for h in range(LH):
                    qn[(sc, h)] = qnp.tile([128, 512], BF16, tag=f"qn{h}",
                                           name=f"qn{h}")
                q_done = 0
                if sc == 0:
                    # dc-major first pass: 6 accumulators fill as the x /
                    # weight DMA waves land, so the PE starts ~1us in
                    # instead of waiting for whole tensors.
                    pa = pu.tile([128, 2, 512], F32, tag="pA", name="pa0")
                    pb = pu.tile([128, 2, 512], F32, tag="pB", name="pb0")
                    k0 = acc_tile(nm="kraw0")
                    k1 = acc_tile(nm="kraw1")
                    tgts = [(k0[:], wk_t, 0), (k1[:], wk_t, 1),
                            (pa[:, 0], wq_t, 0), (pa[:, 1], wq_t, 1),
                            (pb[:, 0], wq_t, 2), (pb[:, 1], wq_t, 3)]
                    for dc in range(NDC):
                        for ap, wt, e in tgts:
                            nc.tensor.matmul(
                                ap, wt[:, dc, e * HD:(e + 1) * HD],
                                xa[:, dc], start=(dc == 0),
                                stop=(dc == NDC - 1))
                    knorm(0, k0[:])
                    knorm(1, k1[:])
                    qnorm(0, pa[:, 0])
                    qnorm(1, pa[:, 1])
                    qnorm(2, pb[:, 0])
                    qnorm(3, pb[:, 1])
                    q_done = 4
                else:
                    for g in range(LKV):
                        kraw = acc_tile(nm=f"kraw{g}")
                        for dc in range(NDC):
                            nc.tensor.matmul(
                                kraw[:], wk_t[:, dc, g * HD:(g + 1) * HD],
                                xa[:, dc], start=(dc == 0),
                                stop=(dc == NDC - 1))
                        knorm(g, kraw[:])
                for h in range(q_done, LH):
                    qraw = acc_tile(nm=f"qraw{h}")
                    for dc in range(NDC):
                        nc.tensor.matmul(
                            qraw[:], wq_t[:, dc, h * HD:(h + 1) * HD],
                            xa[:, dc], start=(dc == 0), stop=(dc == NDC - 1))
                    qnorm(h, qraw[:])
                    nv = 2 if sc == 0 and h % 2 == 1 else \
                        (1 if h % 2 == 1 else 0)
                    for _ in range(nv):
                        st4 = _pv[0]
                        _pv[0] += 1
                        vacc = acc_tile((128, LKV * HD), nm=f"vacc{st4}")
                        for dc in range(NDC):
                            nc.tensor.matmul(
                                vacc[:], xa[:, dc,
                                            (st4 % 4) * 128:
                                            (st4 % 4 + 1) * 128],
                                wv_t[:, dc], start=(dc == 0),
                                stop=(dc == NDC - 1))
                        nc.scalar.copy(v_t[:, st4], vacc[:])

            yt_sb = {}

            def phaseA(qc, filler=()):
                filler = list(filler)
                npair = 2 * (qc + 1)
                # two heads interleaved; same kv group within each duo
                for duo in range(LH // 2):
                    if duo > 0:
                        # out-proj groups of the previous q-chunk: keep the
                        # PE busy across the duo boundary while the scalar
                        # queue drains its exp backlog
                        for _ in range(min(2, len(filler))):
                            filler.pop(0)()
                    hs = (2 * duo, 2 * duo + 1)
                    g = hs[0] // 4
                    yt = {}
                    lps = {}
                    pend = {h: [] for h in hs}
                    for i, h in enumerate(hs):
                        yt[h] = pu.tile([128, 512], F32, tag=f"acc{i}",
                                        name=f"yt{h}")
                        lps[h] = vec1_tile(f"l{h}")

                    def flush(h, last):
                        """yt/l accumulation for the oldest pending pair of
                        head h.  Diagonal chunks (m>=0) only produce columns
                        [128m, 512); remaining columns are covered by other
                        chunks (start=True clears the whole bank; untouched
                        cols get overwritten later via has_written)."""
                        ex, j, first = pend[h].pop(0)
                        sl = []
                        for o in range(2):
                            m = 2 * j + o - 4 * qc
                            sl.append((o, 128 * m if m > 0 else 0))
                        for o, n0 in sl:
                            nc.tensor.matmul(
                                yt[h][:, n0:512],
                                v_t[:, 2 * j + o, g * HD:(g + 1) * HD],
                                ex[:, o, n0:512], start=(first and o == 0),
                                stop=(last and o == 1))
                        for o, n0 in sl:
                            nc.tensor.matmul(
                                lps[h][:, n0:512], o128[:],
                                ex[:, o, n0:512],
                                start=(first and o == 0),
                                stop=(last and o == 1))

                    # diagonal pairs first (smaller masked one leads):
                    # their exp+mask chains are longest
                    order = [2 * qc + 1, 2 * qc] + list(range(2 * qc))
                    for idx, j in enumerate(order):
                        m_off = 2 * j - 4 * qc   # >=0 on diagonal pairs
                        # diag pair (m_off=2,3) only needs columns 256:512
                        c0 = 256 if m_off == 2 else 0
                        for i, h in enumerate(hs):
                            pair = pu.tile([128, 2, 512], F32,
                                           tag=("pA" if i == 0 else "pB"),
                                           name=f"p{h}_{j}")
                            for o in range(2):
                                m = m_off + o
                                n0 = 128 * m if m > 0 else 0
                                nc.tensor.matmul(
                                    pair[:, o, n0:512],
                                    kn_t[:, g, (2 * j + o) * 128:
                                         (2 * j + o + 1) * 128],
                                    qn[(qc, h)][:, n0:512],
                                    start=True, stop=True)
                            ex = exp4.tile([128, 2, 512], BF16, tag="ex",
                                           name="ex")
                            nc.scalar.activation(
                                ex[:, :, c0:512], pair[:, :, c0:512],
                                AF.Exp, scale=gains[:, h:h + 1])
                            if m_off >= 0:
                                # keep iff (n+c0) - 128*(m_off+o) - p >= 0
                                nc.gpsimd.affine_select(
                                    out=ex[:, :, c0:512],
                                    in_=ex[:, :, c0:512],
                                    compare_op=mybir.AluOpType.is_ge,
                                    fill=0.0, base=c0 - 128 * m_off,
                                    pattern=[[-128, 2], [1, 512 - c0]],
                                    channel_multiplier=-1)
                            pend[h].append((ex, j, idx == 0))
                        for h in hs:
                            if len(pend[h]) > 2:
                                flush(h, False)
                    for h in hs:
                        while len(pend[h]) > 1:
                            flush(h, False)
                        flush(h, True)
                        lf = scr1.tile([1, 512], F32, tag="lf", name="lf")
                        nc.vector.reciprocal_approx_fast(out=lf[:],
                                                         in_=lps[h][:])
                        Lsb = scr.tile([128, 512], F32, tag="Lsb", name="Lsb")
                        nc.gpsimd.partition_broadcast(Lsb[:], lf[:])
                        ys = ytp.tile([128, 512], BF16, tag=f"yts{h}",
                                      name=f"yts{h}")
                        yt_sb[(qc, h)] = ys
                        nc.vector.tensor_mul(ys[:], yt[h][:], Lsb[:])

            def o_group(qc, jcol, st4):
                pos0 = qc * 512
                prs = acc_tile(nm=f"pr{jcol}{st4}")
                for h in range(LH):
                    nc.tensor.matmul(
                        prs[:],
                        yt_sb[(qc, h)][:, st4 * 128:(st4 + 1) * 128],
                        wp_t[:, h, jcol * 512:(jcol + 1) * 512],
                        start=(h == 0), stop=(h == LH - 1))
                stg = scr.tile([128, 512], BF16, tag="stg", name="stg")
                nc.vector.tensor_copy(stg[:], prs[:])
                nc.sync.dma_start(
                    out=out[pos0 + st4 * 128:pos0 + (st4 + 1) * 128,
                            jcol * 512:(jcol + 1) * 512],
                    in_=stg[:])

            def o_fillers(qc, n):
                return [(lambda jc=jc, s4=s4: o_group(qc, jc, s4))
                        for jc in range(4) for s4 in range(4)][:n]

            def phaseO(qc, skip=0):
                for i, (jcol, st4) in enumerate(
                        [(jc, s4) for jc in range(4) for s4 in range(4)]):
                    if i >= skip:
                        o_group(qc, jcol, st4)

            def prefetch_x(sc):
                xa = xap.tile([128, NDC, 512], BF16, tag="xa", name="xa")
                dma_x(xa, sc * 512)
                xa_pre[sc] = xa

            phaseP(0)
            phaseP(1)
            phaseA(0)
            prefetch_x(2)
            phaseP(2)
            phaseA(1, o_fillers(0, 6))
            prefetch_x(3)
            phaseO(0, skip=6)
            phaseP(3)
            phaseA(2, o_fillers(1, 6))
            phaseO(1, skip=6)
            phaseA(3, o_fillers(2, 6))
            phaseO(2, skip=6)
            phaseO(3)
    nc.compile()
    return nc


def _rope_tables():
    inv = 1.0 / (10000.0 ** (np.arange(0, ROPE, 2, dtype=np.float64) / ROPE))
    fr = np.outer(np.arange(S, dtype=np.float64), inv)  # [S, 32]
    cos = np.cos(fr).T  # [32, S]
    sin = np.sin(fr).T
    ccat = np.concatenate([cos, cos], axis=0)
    scat = np.concatenate([-sin, sin], axis=0)
    return (ccat.astype(ml_dtypes.bfloat16), scat.astype(ml_dtypes.bfloat16))


def kernel(x, Wq, Wk, Wv, Wproj, q_gain):
    global _cached_program, _last_in_maps
    x = np.asarray(x, dtype=np.float32)
    Wq = np.asarray(Wq, dtype=np.float32)
    Wk = np.asarray(Wk, dtype=np.float32)
    Wv = np.asarray(Wv, dtype=np.float32)
    Wproj = np.asarray(Wproj, dtype=np.float32)
    q_gain = np.asarray(q_gain, dtype=np.float32)

    ccat, scat = _rope_tables()
    o128 = np.ones((128, 1), dtype=ml_dtypes.bfloat16)
    scale = 1.0 / np.sqrt(HD)

    bf = ml_dtypes.bfloat16
    in_maps = []
    for core in range(N_CORES):
        b, half = core // 2, core % 2
        g0 = half * LKV
        gains = np.repeat((q_gain[half * LH:(half + 1) * LH] * scale)
                          [None, :], 128, axis=0).astype(np.float32)
        in_maps.append({
            "xT": np.ascontiguousarray(x[b].T).astype(bf),
            "wqT": np.ascontiguousarray(
                Wq[half * LH * HD:(half + 1) * LH * HD, :].T).astype(bf),
            "wkT": np.ascontiguousarray(
                Wk[g0 * HD:(g0 + LKV) * HD, :].T).astype(bf),
            "wvT": np.ascontiguousarray(
                Wv[g0 * HD:(g0 + LKV) * HD, :].T).astype(bf),
            "wpT": np.ascontiguousarray(
                Wproj[:, half * LH * HD:(half + 1) * LH * HD].T).astype(bf),
            "ccat": ccat, "scat": scat,
            "o128": o128, "gains": gains,
        })

    _last_in_maps = in_maps
    if _cached_program is None:
        _cached_program = _build_program()
    res = run_bass_kernel_spmd(_cached_program, in_maps, list(range(N_CORES)))

    outp = np.empty((B, S, D), dtype=np.float32)
    for b in range(B):
        outp[b] = (res.results[2 * b]["out"].astype(np.float32)
                   + res.results[2 * b + 1]["out"].astype(np.float32))
    return outp


# revision 32
# speedup vs baseline: 1.0108x; 1.0108x over previous
"""Causal self-attention (GQA + RMS-norm + partial RoPE) Trainium2 kernel.

Full inputs in, full output out. Sharding: 8 cores = batch(4) x head-half(2).
Each core handles one batch and 8 q-heads / 2 kv-heads in transposed layouts
(head_dim on partitions). v3 design:

- All matmul operands bf16 (fp32 PSUM accumulate): fast weight load, DVE 2x
  modes, halved SBUF/DMA. Weights DMAed once and kept resident.
- Single activation table set (natural_log_exp_and_others): Exp, Ln, Square,
  Copy only; rsqrt(x) = exp(-0.5*ln(x)). Table chooser pinned to that set.
- R / 1/l broadcasts via gpsimd.partition_broadcast (no tensor-engine
  broadcast matmuls, no PSUM banks for them).
- Attention: key chunks in PAIRS ([128,2,512] PSUM, one Exp per pair);
  TWO heads interleaved so one head's exp/mask latency hides behind the
  other head's matmuls. Causal mask: one gpsimd affine_select per
  diagonal pair.
- PSUM banks: pA/pB [128,2,512] (2 each) + acc0/acc1 [128,512] +
  vec1a/vec1b [1,512] = 8.
"""
import numpy as np
import ml_dtypes

import concourse.bacc as bacc
import concourse.mybir as mybir
import concourse.bass_isa as bass_isa
from concourse.tile import TileContext
from concourse.bass_utils import run_bass_kernel_spmd

# The ACT table-load inserter picks the FIRST act-function set covering each
# activation: Exp/Square/Copy -> exp_and_others(0) but Ln -> natural_log(5),
# so interleaved norm+softmax work thrashes table loads (~1.3us each).  All
# four functions we use coexist in natural_log_exp_and_others; steer the
# chooser there by hiding them from the coverage sets of every OTHER table.
# Set ids are unchanged, so the emitted program stays valid.
_AF = mybir.ActivationFunctionType
_PINNED_SET = "natural_log_exp_and_others"
_PINNED_FUNCS = {_AF.Exp, _AF.Ln, _AF.Square, _AF.Copy}
_orig_get_act_tables = bacc.get_activation_tables


def _pinned_get_act_tables(arch):
    tabs = _orig_get_act_tables(arch)
    return {
        name: (funcs if name == _PINNED_SET else funcs - _PINNED_FUNCS)
        for name, funcs in tabs.items()
    }


bacc.get_activation_tables = _pinned_get_act_tables

F32 = mybir.dt.float32
F32R = mybir.dt.float32r
BF16 = mybir.dt.bfloat16
AF = mybir.ActivationFunctionType

B, S, D = 4, 2048, 2048
H, KV, HD = 16, 4, 128
ROPE, HALF_ROPE = 64, 32
EPS = 1.1920929e-07
N_CORES = 8
NDC = D // 128          # 16 contraction chunks
NQC = S // 512          # 4 query chunks of 512
LH = 8                  # local q heads per core
LKV = 2                 # local kv heads per core

_cached_program = None
_last_in_maps = None


def _build_program():
    nc = bacc.Bacc("TRN2")
    t = nc.alloc_sbuf_tensor("const-f32-eps", [128, 1], F32)
    nc.gpsimd.memset(t.ap(), EPS)
    nc.const_aps.aps[(F32, EPS)] = t.ap()
    nc.all_engine_barrier()

    xT = nc.declare_dram_parameter("xT", [D, S], BF16, isOutput=False)
    wqT = nc.declare_dram_parameter("wqT", [D, LH * HD], BF16, isOutput=False)
    wkT = nc.declare_dram_parameter("wkT", [D, LKV * HD], BF16, isOutput=False)
    wvT = nc.declare_dram_parameter("wvT", [D, LKV * HD], BF16, isOutput=False)
    wpT = nc.declare_dram_parameter("wpT", [LH * HD, D], BF16, isOutput=False)
    ccatd = nc.declare_dram_parameter("ccat", [ROPE, S], BF16, isOutput=False)
    scatd = nc.declare_dram_parameter("scat", [ROPE, S], BF16, isOutput=False)
    o128d = nc.declare_dram_parameter("o128", [128, 1], BF16, isOutput=False)
    gaind = nc.declare_dram_parameter("gains", [128, LH], F32, isOutput=False)
    out = nc.declare_dram_parameter("out", [S, D], BF16, isOutput=True)

    with TileContext(nc) as tc:
        with (
            tc.tile_pool(name="cp", bufs=1) as cp,
            tc.tile_pool(name="xap", bufs=2) as xap,
            tc.tile_pool(name="qnp", bufs=2) as qnp,
            tc.tile_pool(name="ytp", bufs=2) as ytp,
            tc.tile_pool(name="scr", bufs=2) as scr,
            tc.tile_pool(name="exp4", bufs=6) as exp4,
            tc.tile_pool(name="scr1", bufs=1) as scr1,
            tc.tile_pool(name="pu", bufs=1, space="PSUM") as pu,
        ):
            # ---- constants / weights, DMA-ordered by first use ----
            o128 = cp.tile([128, 1], BF16, tag="o128")
            nc.sync.dma_start(out=o128[:], in_=o128d[:])
            gains = cp.tile([128, LH], F32, tag="gains")
            nc.sync.dma_start(out=gains[:], in_=gaind[:])
            ccat = cp.tile([ROPE, S], BF16, tag="ccat")
            nc.sync.dma_start(out=ccat[:], in_=ccatd[:])
            scat = cp.tile([ROPE, S], BF16, tag="scat")
            nc.sync.dma_start(out=scat[:], in_=scatd[:])
            wk_t = cp.tile([128, NDC, LKV * HD], BF16, tag="wk")
            xa0 = xap.tile([128, NDC, 512], BF16, tag="xa", name="xa")
            wq_t = cp.tile([128, NDC, LH * HD], BF16, tag="wq")
            wv_t = cp.tile([128, NDC, LKV * HD], BF16, tag="wv")

            def dma_x(xa, pos0):
                for dc in range(NDC):
                    nc.sync.dma_start(
                        out=xa[:, dc],
                        in_=xT[dc * 128:(dc + 1) * 128, pos0:pos0 + 512])

            for dc in range(NDC):
                nc.sync.dma_start(out=wk_t[:, dc],
                                  in_=wkT[dc * 128:(dc + 1) * 128, :])
                nc.gpsimd.dma_start(out=xa0[:, dc],
                                    in_=xT[dc * 128:(dc + 1) * 128, 0:512])
                nc.sync.dma_start(out=wq_t[:, dc],
                                  in_=wqT[dc * 128:(dc + 1) * 128, :])
            for dc in range(NDC):
                nc.gpsimd.dma_start(out=wv_t[:, dc],
                                    in_=wvT[dc * 128:(dc + 1) * 128, :])
            xa1 = xap.tile([128, NDC, 512], BF16, tag="xa", name="xa")
            dma_x(xa1, 512)
            wp_t = cp.tile([128, LH, D], BF16, tag="wp")
            for hh in range(LH):
                nc.sync.dma_start(out=wp_t[:, hh],
                                  in_=wpT[hh * 128:(hh + 1) * 128, :])
            kn_t = cp.tile([128, LKV, S], BF16, tag="kn")
            v_t = cp.tile([128, S // 128, LKV * HD], BF16, tag="v")
            xa_pre = {0: xa0, 1: xa1}

            # PSUM accumulator rotation: acc0, acc1, plus the (wider) pA/pB
            # slots which the P/O phases may borrow as [128,512] tiles.
            _rot = [0]
            _ROT_TAGS = ["acc0", "acc1", "pA", "pB"]

            def acc_tile(shape=(128, 512), nm="acc"):
                tag = _ROT_TAGS[_rot[0] % 4]
                _rot[0] += 1
                return pu.tile(list(shape), F32, tag=tag, name=nm)

            _v1 = [0]

            def vec1_tile(nm):
                tag = "vec1a" if _v1[0] % 2 == 0 else "vec1b"
                _v1[0] += 1
                return pu.tile([1, 512], F32, tag=tag, name=nm)

            def norm_rope(raw, dst_full, dst_r1, dst_r2, dst_r64, cs):
                """RMS-norm + partial RoPE, raw [128,512] PSUM f32 ->
                bf16 dst (already-allocated APs: full/rows0:32/32:64/0:64).
                cs = column slice into the S-wide rope tables."""
                sq = scr.tile([128, 512], BF16, tag="sq", name="sq")
                nc.scalar.activation(sq[:], raw, AF.Square)
                ssq = vec1_tile("ssq")
                nc.tensor.matmul(ssq[:], o128[:], sq[:], start=True, stop=True)
                lnu = scr1.tile([1, 512], F32, tag="lnu", name="lnu")
                nc.scalar.activation(lnu[:], ssq[:], AF.Ln,
                                     scale=1.0 / HD, bias=EPS)
                rr = scr1.tile([1, 512], F32, tag="rr", name="rr")
                nc.scalar.activation(rr[:], lnu[:], AF.Exp, scale=-0.5)
                Rb = scr.tile([128, 512], F32, tag="Rb", name="Rb")
                nc.gpsimd.partition_broadcast(Rb[:], rr[:])
                nc.vector.tensor_mul(dst_full, raw, Rb[:])
                # scat rows 0:32 = -sin, rows 32:64 = +sin so each TT below
                # has equal SBUF base partitions for its two inputs
                tmp = scr.tile([ROPE, 512], BF16, tag="tmp", name="tmp")
                nc.vector.tensor_mul(tmp[0:HALF_ROPE, :], dst_r2,
                                     scat[HALF_ROPE:ROPE, cs])
                nc.vector.tensor_mul(tmp[HALF_ROPE:ROPE, :], dst_r1,
                                     scat[0:HALF_ROPE, cs])
                nc.vector.tensor_mul(dst_r64, dst_r64, ccat[:, cs])
                nc.vector.tensor_add(dst_r64, dst_r64, tmp[:])

            qn = {}
            _pv = [0]

            def phaseP(sc):
                pos0 = sc * 512
                cs = slice(pos0, pos0 + 512)
                xa = xa_pre.pop(sc)
                def knorm(g, raw_ap):
                    norm_rope(raw_ap, kn_t[:, g, cs],
                              kn_t[0:HALF_ROPE, g, cs],
                              kn_t[HALF_ROPE:ROPE, g, cs],
                              kn_t[0:ROPE, g, cs], cs)

                def qnorm(h, raw_ap):
                    qt = qn[(sc, h)]
                    norm_rope(raw_ap, qt[:], qt[0:HALF_ROPE, :],
                              qt[HALF_ROPE:ROPE, :], qt[0:ROPE, :], cs)

                for h in range(LH):
                    qn[(sc, h)] = qnp.tile([128, 512], BF16, tag=f"qn{h}",
                                           name=f"qn{h}")
                q_done = 0
                if sc == 0:
                    # dc-major first pass: 6 accumulators fill as the x /
                    # weight DMA waves land, so the PE starts ~1us in
                    # instead of waiting for whole tensors.
                    pa = pu.tile([128, 2, 512], F32, tag="pA", name="pa0")
                    pb = pu.tile([128, 2, 512], F32, tag="pB", name="pb0")
                    k0 = acc_tile(nm="kraw0")
                    k1 = acc_tile(nm="kraw1")
                    tgts = [(k0[:], wk_t, 0), (k1[:], wk_t, 1),
                            (pa[:, 0], wq_t, 0), (pa[:, 1], wq_t, 1),
                            (pb[:, 0], wq_t, 2), (pb[:, 1], wq_t, 3)]
                    for dc in range(NDC):
                        for ap, wt, e in tgts:
                            nc.tensor.matmul(
                                ap, wt[:, dc, e * HD:(e + 1) * HD],
                                xa[:, dc], start=(dc == 0),
                                stop=(dc == NDC - 1))
                    knorm(0, k0[:])
                    knorm(1, k1[:])
                    qnorm(0, pa[:, 0])
                    qnorm(1, pa[:, 1])
                    qnorm(2, pb[:, 0])
                    qnorm(3, pb[:, 1])
                    q_done = 4
                else:
                    for g in range(LKV):
                        kraw = acc_tile(nm=f"kraw{g}")
                        for dc in range(NDC):
                            nc.tensor.matmul(
                                kraw[:], wk_t[:, dc, g * HD:(g + 1) * HD],
                                xa[:, dc], start=(dc == 0),
                                stop=(dc == NDC - 1))
                        knorm(g, kraw[:])
                for h in range(q_done, LH):
                    qraw = acc_tile(nm=f"qraw{h}")
                    for dc in range(NDC):
                        nc.tensor.matmul(
                            qraw[:], wq_t[:, dc, h * HD:(h + 1) * HD],
                            xa[:, dc], start=(dc == 0), stop=(dc == NDC - 1))
                    qnorm(h, qraw[:])
                    nv = 2 if sc == 0 and h % 2 == 1 else \
                        (1 if h % 2 == 1 else 0)
                    for _ in range(nv):
                        st4 = _pv[0]
                        _pv[0] += 1
                        vacc = acc_tile((128, LKV * HD), nm=f"vacc{st4}")
                        for dc in range(NDC):
                            nc.tensor.matmul(
                                vacc[:], xa[:, dc,
                                            (st4 % 4) * 128:
                                            (st4 % 4 + 1) * 128],
                                wv_t[:, dc], start=(dc == 0),
                                stop=(dc == NDC - 1))
                        nc.scalar.copy(v_t[:, st4], vacc[:])

            yt_sb = {}

            def phaseA(qc, filler=()):
                filler = list(filler)
                npair = 2 * (qc + 1)
                # two heads interleaved; same kv group within each duo
                for duo in range(LH // 2):
                    if duo > 0:
                        # out-proj groups of the previous q-chunk: keep the
                        # PE busy across the duo boundary while the scalar
                        # queue drains its exp backlog
                        for _ in range(min(2, len(filler))):
                            filler.pop(0)()
                    hs = (2 * duo, 2 * duo + 1)
                    g = hs[0] // 4
                    yt = {}
                    lps = {}
                    pend = {h: [] for h in hs}
                    for i, h in enumerate(hs):
                        yt[h] = pu.tile([128, 512], F32, tag=f"acc{i}",
                                        name=f"yt{h}")
                        lps[h] = vec1_tile(f"l{h}")

                    def flush(h, last):
                        """yt/l accumulation for the oldest pending pair of
                        head h.  Diagonal chunks (m>=0) only produce columns
                        [128m, 512); remaining columns are covered by other
                        chunks (start=True clears the whole bank; untouched
                        cols get overwritten later via has_written)."""
                        ex, j, first = pend[h].pop(0)
                        sl = []
                        for o in range(2):
                            m = 2 * j + o - 4 * qc
                            sl.append((o, 128 * m if m > 0 else 0))
                        for o, n0 in sl:
                            nc.tensor.matmul(
                                yt[h][:, n0:512],
                                v_t[:, 2 * j + o, g * HD:(g + 1) * HD],
                                ex[:, o, n0:512], start=(first and o == 0),
                                stop=(last and o == 1))
                        for o, n0 in sl:
                            nc.tensor.matmul(
                                lps[h][:, n0:512], o128[:],
                                ex[:, o, n0:512],
                                start=(first and o == 0),
                                stop=(last and o == 1))

                    # diagonal pairs first (smaller masked one leads):
                    # their exp+mask chains are longest
                    order = [2 * qc + 1, 2 * qc] + list(range(2 * qc))
                    for idx, j in enumerate(order):
                        m_off = 2 * j - 4 * qc   # >=0 on diagonal pairs
                        # diag pair (m_off=2,3) only needs columns 256:512
                        c0 = 256 if m_off == 2 else 0
                        for i, h in enumerate(hs):
                            pair = pu.tile([128, 2, 512], F32,
                                           tag=("pA" if i == 0 else "pB"),
                                           name=f"p{h}_{j}")
                            for o in range(2):
                                m = m_off + o
                                n0 = 128 * m if m > 0 else 0
                                nc.tensor.matmul(
                                    pair[:, o, n0:512],
                                    kn_t[:, g, (2 * j + o) * 128:
                                         (2 * j + o + 1) * 128],
                                    qn[(qc, h)][:, n0:512],
                                    start=True, stop=True)
                            ex = exp4.tile([128, 2, 512], BF16, tag="ex",
                                           name="ex")
                            nc.scalar.activation(
                                ex[:, :, c0:512], pair[:, :, c0:512],
                                AF.Exp, scale=gains[:, h:h + 1])
                            if m_off >= 0:
                                # keep iff (n+c0) - 128*(m_off+o) - p >= 0
                                nc.gpsimd.affine_select(
                                    out=ex[:, :, c0:512],
                                    in_=ex[:, :, c0:512],
                                    compare_op=mybir.AluOpType.is_ge,
                                    fill=0.0, base=c0 - 128 * m_off,
                                    pattern=[[-128, 2], [1, 512 - c0]],
                                    channel_multiplier=-1)
                            pend[h].append((ex, j, idx == 0))
                        for h in hs:
                            if len(pend[h]) > 2:
                                flush(h, False)
                    for h in hs:
                        while len(pend[h]) > 1:
                            flush(h, False)
                        flush(h, True)
                        lf = scr1.tile([1, 512], F32, tag="lf", name="lf")
                        nc.vector.reciprocal_approx_fast(out=lf[:],
                                                         in_=lps[h][:])
                        Lsb = scr.tile([128, 512], F32, tag="Lsb", name="Lsb")
                        nc.gpsimd.partition_broadcast(Lsb[:], lf[:])
                        ys = ytp.tile([128, 512], BF16, tag=f"yts{h}",
                                      name=f"yts{h}")
                        yt_sb[(qc, h)] = ys
                        nc.vector.tensor_mul(ys[:], yt[h][:], Lsb[:])

            def o_group(qc, jcol, st4):
                pos0 = qc * 512
                prs = acc_tile(nm=f"pr{jcol}{st4}")
                for h in range(LH):
                    nc.tensor.matmul(
                        prs[:],
                        yt_sb[(qc, h)][:, st4 * 128:(st4 + 1) * 128],
                        wp_t[:, h, jcol * 512:(jcol + 1) * 512],
                        start=(h == 0), stop=(h == LH - 1))
                stg = scr.tile([128, 512], BF16, tag="stg", name="stg")
                nc.vector.tensor_copy(stg[:], prs[:])
                nc.sync.dma_start(
                    out=out[pos0 + st4 * 128:pos0 + (st4 + 1) * 128,
                            jcol * 512:(jcol + 1) * 512],
                    in_=stg[:])

            def o_fillers(qc, n):
                return [(lambda jc=jc, s4=s4: o_group(qc, jc, s4))
                        for jc in range(4) for s4 in range(4)][:n]

            def phaseO(qc, skip=0):
                for i, (jcol, st4) in enumerate(
                        [(jc, s4) for jc in range(4) for s4 in range(4)]):
                    if i >= skip:
                        o_group(qc, jcol, st4)

            def prefetch_x(sc):
                xa = xap.tile([128, NDC, 512], BF16, tag="xa", name="xa")
                dma_x(xa, sc * 512)
                xa_pre[sc] = xa

            phaseP(0)
            phaseP(1)
            phaseA(0)
            prefetch_x(2)
            phaseP(2)
            phaseA(1, o_fillers(0, 6))
            prefetch_x(3)
            phaseO(0, skip=6)
            phaseP(3)
            phaseA(2, o_fillers(1, 6))
            phaseO(1, skip=6)
            phaseA(3, o_fillers(2, 6))
            phaseO(2, skip=6)
            phaseO(3)
    nc.compile()
    return nc


def _rope_tables():
    inv = 1.0 / (10000.0 ** (np.arange(0, ROPE, 2, dtype=np.float64) / ROPE))
    fr = np.outer(np.arange(S, dtype=np.float64), inv)  # [S, 32]
    cos = np.cos(fr).T  # [32, S]
    sin = np.sin(fr).T
    ccat = np.concatenate([cos, cos], axis=0)
    scat = np.concatenate([-sin, sin], axis=0)
    return (ccat.astype(ml_dtypes.bfloat16), scat.astype(ml_dtypes.bfloat16))


def kernel(x, Wq, Wk, Wv, Wproj, q_gain):
    global _cached_program, _last_in_maps
    x = np.asarray(x, dtype=np.float32)
    Wq = np.asarray(Wq, dtype=np.float32)
    Wk = np.asarray(Wk, dtype=np.float32)
    Wv = np.asarray(Wv, dtype=np.float32)
    Wproj = np.asarray(Wproj, dtype=np.float32)
    q_gain = np.asarray(q_gain, dtype=np.float32)

    ccat, scat = _rope_tables()
    o128 = np.ones((128, 1), dtype=ml_dtypes.bfloat16)
    scale = 1.0 / np.sqrt(HD)

    bf = ml_dtypes.bfloat16
    in_maps = []
    for core in range(N_CORES):
        b, half = core // 2, core % 2
        g0 = half * LKV
        gains = np.repeat((q_gain[half * LH:(half + 1) * LH] * scale)
                          [None, :], 128, axis=0).astype(np.float32)
        in_maps.append({
            "xT": np.ascontiguousarray(x[b].T).astype(bf),
            "wqT": np.ascontiguousarray(
                Wq[half * LH * HD:(half + 1) * LH * HD, :].T).astype(bf),
            "wkT": np.ascontiguousarray(
                Wk[g0 * HD:(g0 + LKV) * HD, :].T).astype(bf),
            "wvT": np.ascontiguousarray(
                Wv[g0 * HD:(g0 + LKV) * HD, :].T).astype(bf),
            "wpT": np.ascontiguousarray(
                Wproj[:, half * LH * HD:(half + 1) * LH * HD].T).astype(bf),
            "ccat": ccat, "scat": scat,
            "o128": o128, "gains": gains,
        })

    _last_in_maps = in_maps
    if _cached_program is None:
        _cached_program = _build_program()
    res = run_bass_kernel_spmd(_cached_program, in_maps, list(range(N_CORES)))

    outp = np.empty((B, S, D), dtype=np.float32)
    for b in range(B):
        outp[b] = (res.results[2 * b]["out"].astype(np.float32)
                   + res.results[2 * b + 1]["out"].astype(np.float32))
    return outp
